# revision 1
# baseline (speedup 1.0000x reference)
"""Self-contained Trainium2 (Bass) kernel for the 2-layer GCN + MLP model.

Strategy (node-parallel, dst-sharded, two SPMD launches):
  * Host prep (index ops only): CSR-sort edges by dst, shard nodes over the 8
    cores, bucket each core's nodes by in-degree, give every node a fixed
    number of edge slots (bucket stride).  Edge streams are host-gathered into
    the slot layout; padding slots carry zeros.
  * Launch A (per core): w = rsqrt(deg[src]); y = x[src]*w; dense fixed-stride
    reduce over slots -> agg; z1' = dinv*(agg + dinv*x);
    g2 = relu([dinv*z1', dinv] @ [W1;b1])  (the dinv-scaled layer-1 output,
    i.e. the complete layer-2 message per node), written out per core.
  * Host: concatenates the per-core g2 slices and gathers g2[src] into the
    slot layout for each core (pure index-space data movement).
  * Launch B (per core): dense fixed-stride reduce of the g2 slot stream ->
    agg2; z2 = dinv*(agg2 + g2_own); then the MLP chain with weights baked as
    immediates: sigmoid(.W2+b2) -> relu(.W3+b3) -> relu(.W4+b4) -> .W5+b5.
  * Host: unpermute per-core outputs back to original node order.

All floating-point math runs on device; the host only sorts, indexes, pads
and concatenates.
"""
import numpy as np

import concourse.bass as bass
from concourse.bacc import Bacc
import concourse.mybir as mybir
import concourse.tile as tile

NCORES = 8
N = 1_000_000
P = 128
F32 = mybir.dt.float32
BF16 = mybir.dt.bfloat16
U8 = mybir.dt.uint8
AF = mybir.ActivationFunctionType
OP = mybir.AluOpType


# ----------------------------------------------------------------- host prep
def _choose_strides(max_deg):
    ss = [s for s in (2, 4, 6, 8, 10, 12, 14, 16, 20, 24, 28, 32, 36, 40, 48,
                      64, 96, 128, 192, 256, 384, 512) if s < max_deg]
    ss.append(int(max_deg))
    return ss


def _prep(x, edge_index, ncores=NCORES, n=N):
    npc = n // ncores
    src = np.asarray(edge_index[0]).astype(np.int64)
    dst = np.asarray(edge_index[1]).astype(np.int64)
    deg_in = np.bincount(dst, minlength=n)
    strides = _choose_strides(max(int(deg_in.max()), 2))
    strides_arr = np.asarray(strides)
    nb = len(strides)

    order = np.argsort(dst, kind="stable")
    src_s = src[order]
    rowptr = np.zeros(n + 1, dtype=np.int64)
    np.cumsum(deg_in, out=rowptr[1:])

    bucket_of = np.searchsorted(strides_arr, deg_in)
    bucket_of[deg_in == 0] = -1

    m_b = np.zeros((ncores, nb), dtype=np.int64)
    node_lists = [[None] * nb for _ in range(ncores)]
    for c in range(ncores):
        lo, hi = c * npc, (c + 1) * npc
        nodes_c = np.arange(lo, hi)
        bk = bucket_of[lo:hi]
        for b in range(nb):
            nl = nodes_c[bk == b]
            node_lists[c][b] = nl
            m_b[c, b] = -(-len(nl) // P)
    m_pad = m_b.max(axis=0)
    n_deg0 = max(int((deg_in[c * npc:(c + 1) * npc] == 0).sum())
                 for c in range(ncores))
    m0 = -(-max(n_deg0, 1) // P)
    SUM_M_raw = int(m_pad.sum()) + m0
    SUM_M = -(-SUM_M_raw // 32) * 32
    m0 += SUM_M - SUM_M_raw
    NPCP = P * SUM_M
    boff = np.concatenate([[0], np.cumsum(m_pad)]).astype(np.int64)
    boff0 = int(m_pad.sum())
    SLOTS = int((m_pad * P * strides_arr).sum())

    def make_plan(target):
        cp = []
        for b in range(nb):
            s = strides[b]
            if m_pad[b] == 0:
                continue
            mc = max(32, -(-max(1, target // s) // 32) * 32)
            i = 0
            while i < m_pad[b]:
                take = int(min(mc, m_pad[b] - i))
                cp.append((b, s, int(i), take))
                i += take
        return cp
    chunk_plan = make_plan(4096)
    chunk_plan_B = make_plan(2048)

    storage = np.empty(n, dtype=np.int64)
    origin = np.full((ncores, NPCP), -1, dtype=np.int64)
    for c in range(ncores):
        lo, hi = c * npc, (c + 1) * npc
        deg0_nodes = np.arange(lo, hi)[deg_in[lo:hi] == 0]
        for b in range(nb + 1):
            if b < nb:
                nl, mb, off = node_lists[c][b], int(m_pad[b]), int(boff[b])
            else:
                nl, mb, off = deg0_nodes, m0, boff0
            if len(nl) == 0 or mb == 0:
                continue
            j = np.arange(len(nl))
            p, i = j // mb, j % mb
            sid = p * SUM_M + off + i
            storage[nl] = c * NPCP + sid
            origin[c, sid] = nl

    per_core = []
    for c in range(ncores):
        xg = np.zeros((SLOTS * 2,), dtype=np.float32)
        degS = np.ones((SLOTS,), dtype=np.uint8)
        idxs = np.full((SLOTS,), ncores * NPCP, dtype=np.int64)  # pad row
        sbase = 0
        for b in range(nb):
            s, mb = strides[b], int(m_pad[b])
            if mb == 0:
                continue
            nl = node_lists[c][b]
            if len(nl) > 0:
                j = np.arange(len(nl))
                p, i = j // mb, j % mb
                deg = deg_in[nl]
                node_rep = np.repeat(j, deg)
                k_in = np.arange(len(node_rep)) - np.repeat(
                    np.concatenate([[0], np.cumsum(deg)[:-1]]), deg)
                e_pos = np.repeat(rowptr[nl], deg) + k_in
                slot = sbase + p[node_rep] * (mb * s) + i[node_rep] * s + k_in
                sv = src_s[e_pos]
                # f-major slot position for xg: [p][i][f][k]
                slot_fm = sbase * 2 + (p[node_rep] * mb + i[node_rep]) * (2 * s) + k_in
                xg[slot_fm] = x[sv, 0]
                xg[slot_fm + s] = x[sv, 1]
                degS[slot] = np.minimum(deg_in[sv] + 1, 255).astype(np.uint8)
                idxs[slot] = storage[sv]
            sbase += P * mb * s
        assert sbase == SLOTS

        x_own = np.zeros((2, NPCP), dtype=np.float32)
        deg_own = np.ones((NPCP,), dtype=np.float32)
        valid = origin[c] >= 0
        ov = origin[c][valid]
        x_own[0, valid] = x[ov, 0]
        x_own[1, valid] = x[ov, 1]
        deg_own[valid] = (deg_in[ov] + 1).astype(np.float32)
        per_core.append(dict(xg=xg, degS=degS, idxs=idxs,
                             x_own=x_own, deg_own=deg_own))

    meta = dict(strides=strides, m_pad=m_pad, SUM_M=SUM_M, NPCP=NPCP,
                boff=boff, SLOTS=SLOTS, chunk_plan=chunk_plan,
                chunk_plan_B=chunk_plan_B, origin=origin,
                ncores=ncores, n=n)
    return per_core, meta


def _uncovered_ranges(meta):
    SUM_M = meta["SUM_M"]
    done = np.zeros(SUM_M, dtype=bool)
    for (b, s, i0, mc) in meta["chunk_plan"]:
        j0 = int(meta["boff"][b]) + i0
        done[j0:j0 + mc] = True
    out = []
    jj = 0
    while jj < SUM_M:
        if done[jj]:
            jj += 1
            continue
        j1 = jj
        while j1 < SUM_M and not done[j1]:
            j1 += 1
        out.append((jj, j1))
        jj = j1
    return out


# --------------------------------------------------------- device build: A
def _build_A(meta, W1b, reps=1):
    SUM_M, SLOTS, NPCP = meta["SUM_M"], meta["SLOTS"], meta["NPCP"]
    strides, m_pad, boff = meta["strides"], meta["m_pad"], meta["boff"]
    plan = meta["chunk_plan"]

    nc = Bacc(num_devices=meta["ncores"])
    xg = nc.declare_dram_parameter("xg", [SLOTS * 2], F32, isOutput=False)
    degS = nc.declare_dram_parameter("degS", [SLOTS], U8, isOutput=False)
    x_own = nc.declare_dram_parameter("x_own", [2, NPCP], F32, isOutput=False)
    deg_own = nc.declare_dram_parameter("deg_own", [NPCP], F32, isOutput=False)
    g2out = nc.declare_dram_parameter("g2out", [P, SUM_M, 4], BF16, isOutput=True)

    sbases = {}
    sb = 0
    for b, s in enumerate(strides):
        sbases[b] = sb
        sb += P * int(m_pad[b]) * s

    with tile.TileContext(nc) as tc:
        with tc.tile_pool(name="res", bufs=1) as res:
            dinv = res.tile([P, SUM_M], F32, tag="dinv")
            g2acc = res.tile([P, SUM_M, 4], F32, tag="g2acc")
            xow = res.tile([P, 2, SUM_M], F32, tag="xow")
            for _ in range(reps):
                nc.sync.dma_start(out=dinv[:],
                                  in_=deg_own[:].rearrange("(p j) -> p j", p=P))
                nc.vector.reciprocal(out=dinv[:], in_=dinv[:])
                nc.scalar.activation(out=dinv[:], in_=dinv[:], func=AF.Sqrt)
                nc.sync.dma_start(out=xow[:],
                                  in_=x_own[:].rearrange("f (p j) -> p f j", p=P))

                with tc.tile_pool(name="l1", bufs=2) as st:
                    def g2_cols(z0, z1, dv, j0, mc):
                        """g2acc[:, j0:j0+mc, o] = relu(z0 W[0,o]+z1 W[1,o]+dv W[2,o])"""
                        sl = g2acc[:, j0:j0 + mc, :]
                        for o in range(4):
                            nc.vector.tensor_scalar_mul(
                                out=sl[:, :, o], in0=z0[:], scalar1=float(W1b[0, o]))
                            nc.vector.scalar_tensor_tensor(
                                out=sl[:, :, o], in0=z1[:], scalar=float(W1b[1, o]),
                                in1=sl[:, :, o], op0=OP.mult, op1=OP.add)
                            nc.vector.scalar_tensor_tensor(
                                out=sl[:, :, o], in0=dv, scalar=float(W1b[2, o]),
                                in1=sl[:, :, o], op0=OP.mult, op1=OP.add)
                        nc.scalar.activation(out=sl, in_=sl, func=AF.Relu)

                    for (b, s, i0, mc) in plan:
                        mb = int(m_pad[b])
                        xv = xg[2 * sbases[b]:2 * (sbases[b] + P * mb * s)] \
                            .rearrange("(p i fk) -> p i fk", p=P, i=mb)[:, i0:i0 + mc, :]
                        dv = degS[sbases[b]:sbases[b] + P * mb * s] \
                            .rearrange("(p i k) -> p i k", p=P, i=mb, k=s)[:, i0:i0 + mc, :]
                        xt = st.tile([P, mc, 2 * s], F32, tag="xg")
                        wu = st.tile([P, mc, s], U8, tag="wu")
                        wt = st.tile([P, mc, s], F32, tag="w")
                        nc.sync.dma_start(out=xt[:], in_=xv)
                        nc.sync.dma_start(out=wu[:], in_=dv)
                        nc.vector.reciprocal(out=wt[:], in_=wu[:])
                        nc.scalar.activation(out=wt[:], in_=wt[:], func=AF.Sqrt)
                        j0 = int(boff[b]) + i0
                        dsl = dinv[:, j0:j0 + mc]
                        zf = []
                        for f in range(2):
                            yf = st.tile([P, mc, s], F32, tag="y", name=f"y{f}")
                            nc.vector.tensor_tensor(out=yf[:],
                                                    in0=xt[:, :, f * s:(f + 1) * s],
                                                    in1=wt[:], op=OP.mult)
                            af = st.tile([P, mc], F32, tag=f"agg{f}")
                            nc.vector.tensor_reduce(out=af[:], in_=yf[:],
                                                    axis=mybir.AxisListType.X, op=OP.add)
                            xot = st.tile([P, mc], F32, tag=f"xo{f}")
                            nc.vector.tensor_tensor(out=xot[:], in0=xow[:, f, j0:j0 + mc],
                                                    in1=dsl, op=OP.mult)
                            nc.vector.tensor_tensor(out=af[:], in0=af[:], in1=xot[:], op=OP.add)
                            nc.vector.tensor_tensor(out=af[:], in0=af[:], in1=dsl, op=OP.mult)
                            nc.vector.tensor_tensor(out=af[:], in0=af[:], in1=dsl, op=OP.mult)
                            zf.append(af)
                        g2_cols(zf[0], zf[1], dsl, j0, mc)

                    # uncovered (deg-0 / pad) nodes: agg = 0 -> z1''_f = dinv^3 x_f
                    for (j0, j1) in _uncovered_ranges(meta):
                        mcr = j1 - j0
                        zf = []
                        for f in range(2):
                            xot = st.tile([P, mcr], F32, tag=f"xo{f}")
                            nc.vector.tensor_tensor(out=xot[:], in0=xow[:, f, j0:j1],
                                                    in1=dinv[:, j0:j1], op=OP.mult)
                            for _r in range(2):
                                nc.vector.tensor_tensor(out=xot[:], in0=xot[:],
                                                        in1=dinv[:, j0:j1], op=OP.mult)
                            zf.append(xot)
                        g2_cols(zf[0], zf[1], dinv[:, j0:j1], j0, j1 - j0)

                nc.gpsimd.dma_start(out=g2out[:], in_=g2acc[:])
    return nc


# --------------------------------------------------------- device build: B
def _build_B(meta, weights, reps=1):
    SUM_M, SLOTS, NPCP = meta["SUM_M"], meta["SLOTS"], meta["NPCP"]
    strides, m_pad, boff = meta["strides"], meta["m_pad"], meta["boff"]
    W2, b2 = weights["W2"], weights["b2"]
    W3, b3 = weights["W3"], weights["b3"]
    W4, b4 = weights["W4"], weights["b4"]
    W5, b5 = weights["W5"], weights["b5"]

    plan = meta["chunk_plan_B"]
    nc = Bacc(num_devices=meta["ncores"])
    gs = nc.declare_dram_parameter("gs", [SLOTS * 4], BF16, isOutput=False)
    g2own = nc.declare_dram_parameter("g2own", [P, SUM_M, 4], BF16, isOutput=False)
    deg_own = nc.declare_dram_parameter("deg_own", [NPCP], F32, isOutput=False)
    out = nc.declare_dram_parameter("out", [P, SUM_M], F32, isOutput=True)

    sbases = {}
    sb = 0
    for b, s in enumerate(strides):
        sbases[b] = sb
        sb += P * int(m_pad[b]) * s

    with tile.TileContext(nc) as tc:
        with tc.tile_pool(name="res", bufs=1) as res:
            dinv = res.tile([P, SUM_M], F32, tag="dinv")
            gown = res.tile([P, SUM_M, 4], BF16, tag="gown")
            z2 = [res.tile([P, SUM_M], F32, tag=f"z2_{f}", name=f"z2_{f}")
                  for f in range(4)]
            for _ in range(reps):
                nc.sync.dma_start(out=dinv[:],
                                  in_=deg_own[:].rearrange("(p j) -> p j", p=P))
                nc.vector.reciprocal(out=dinv[:], in_=dinv[:])
                nc.scalar.activation(out=dinv[:], in_=dinv[:], func=AF.Sqrt)
                nc.sync.dma_start(out=gown[:], in_=g2own[:])

                with tc.tile_pool(name="l2", bufs=2) as st:
                    for (b, s, i0, mc) in plan:
                        mb = int(m_pad[b])
                        gv = gs[4 * sbases[b]:4 * (sbases[b] + P * mb * s)] \
                            .rearrange("(p i fk) -> p i fk", p=P, i=mb)[:, i0:i0 + mc, :]
                        gt = st.tile([P, mc, 4 * s], BF16, tag="gath")
                        nc.sync.dma_start(out=gt[:], in_=gv)
                        a2 = st.tile([P, mc, 4], F32, tag="agg2")
                        nc.vector.tensor_reduce(
                            out=a2[:],
                            in_=gt[:].rearrange("p i (f k) -> p i f k", k=s),
                            axis=mybir.AxisListType.X, op=OP.add)
                        j0 = int(boff[b]) + i0
                        dsl = dinv[:, j0:j0 + mc]
                        for f in range(4):
                            zb = z2[f][:, j0:j0 + mc]
                            nc.vector.tensor_tensor(out=zb, in0=a2[:, :, f],
                                                    in1=gown[:, j0:j0 + mc, f], op=OP.add)
                            nc.vector.tensor_tensor(out=zb, in0=zb, in1=dsl, op=OP.mult)
                    for (j0, j1) in _uncovered_ranges(meta):
                        for f in range(4):
                            nc.vector.tensor_tensor(out=z2[f][:, j0:j1],
                                                    in0=gown[:, j0:j1, f],
                                                    in1=dinv[:, j0:j1], op=OP.mult)

                # MLP with immediates; biases via memset tiles
                def dense(ins_, Wm, bias, func, tagp, och):
                    outs_ = []
                    for o in range(och):
                        acc = res.tile([P, SUM_M], F32, tag=f"{tagp}{o}",
                                       name=f"{tagp}{o}")
                        bt = res.tile([P, 1], F32, tag=f"{tagp}b{o}",
                                      name=f"{tagp}b{o}")
                        nc.vector.memset(bt[:], float(bias[o]))
                        nc.vector.tensor_scalar_mul(out=acc[:], in0=ins_[0][:],
                                                    scalar1=float(Wm[0, o]))
                        for i in range(1, len(ins_)):
                            nc.vector.scalar_tensor_tensor(
                                out=acc[:], in0=ins_[i][:], scalar=float(Wm[i, o]),
                                in1=acc[:], op0=OP.mult, op1=OP.add)
                        nc.scalar.activation(out=acc[:], in_=acc[:], func=func,
                                             bias=bt[:])
                        outs_.append(acc)
                    return outs_

                h = dense(z2, W2, b2, AF.Sigmoid, "h2_", 3)
                h = dense(h, W3, b3, AF.Relu, "h3_", 4)
                h = dense(h, W4, b4, AF.Relu, "h4_", 3)
                h = dense(h, W5, b5, AF.Identity, "h5_", 1)
                nc.sync.dma_start(out=out[:], in_=h[0][:])
    return nc


# ------------------------------------------------------------------ driver
def _run_spmd(nc, in_maps, ncores):
    from concourse.bass_utils import run_bass_kernel_spmd
    if not nc.is_finalized():
        nc.finalize()
    return run_bass_kernel_spmd(nc, in_maps, core_ids=list(range(ncores)))


def host_gather_g2(meta, per_core, g2_slices):
    """g2_slices[c]: [P, SUM_M, 4] from launch A. Returns per-core slot
    streams [SLOTS*4] in f-major slot layout (index-space gather only)."""
    NC, NPCP = meta["ncores"], meta["NPCP"]
    strides, m_pad = meta["strides"], meta["m_pad"]
    g2_full = np.concatenate(
        [np.asarray(g2_slices[c]).reshape(NPCP, 4) for c in range(NC)] +
        [np.zeros((1, 4), np.asarray(g2_slices[0]).dtype)], axis=0)
    out = []
    for c in range(NC):
        g = g2_full[per_core[c]["idxs"]]          # [SLOTS, 4] slot-major
        fm = np.empty((meta["SLOTS"] * 4,), dtype=g2_full.dtype)
        sbase = 0
        for b, s in enumerate(strides):
            mb = int(m_pad[b])
            if mb == 0:
                continue
            nseg = P * mb * s
            seg = g[sbase:sbase + nseg].reshape(P * mb, s, 4)
            fm[sbase * 4:(sbase + nseg) * 4] = \
                seg.transpose(0, 2, 1).reshape(-1)
            sbase += nseg
        out.append(fm)
    return out


def kernel(x, edge_index, W1, b1, W2, b2, W3, b3, W4, b4, W5, b5):
    x = np.asarray(x, dtype=np.float32)
    per_core, meta = _prep(x, edge_index)
    W1b = np.concatenate([np.asarray(W1), np.asarray(b1)[None, :]], axis=0)
    weights = dict(W2=np.asarray(W2), b2=np.asarray(b2),
                   W3=np.asarray(W3), b3=np.asarray(b3),
                   W4=np.asarray(W4), b4=np.asarray(b4),
                   W5=np.asarray(W5), b5=np.asarray(b5))
    NC = meta["ncores"]

    ncA = _build_A(meta, W1b)
    resA = _run_spmd(ncA, [{k: d[k] for k in ("xg", "degS", "x_own", "deg_own")}
                           for d in per_core], NC)
    g2_slices = [resA.results[c]["g2out"] for c in range(NC)]

    gs = host_gather_g2(meta, per_core, g2_slices)
    ncB = _build_B(meta, weights)
    resB = _run_spmd(ncB, [dict(gs=gs[c], g2own=np.asarray(g2_slices[c]),
                                deg_own=per_core[c]["deg_own"])
                           for c in range(NC)], NC)

    full = np.zeros(meta["n"], dtype=np.float32)
    for c in range(NC):
        o = np.asarray(resB.results[c]["out"]).reshape(-1)
        org = meta["origin"][c]
        valid = org >= 0
        full[org[valid]] = o[valid]
    return full



# revision 7
# speedup vs baseline: 15.6690x; 15.6690x over previous
"""Self-contained Trainium2 (Bass) kernel for the 2-layer GCN + MLP model.

Strategy (node-parallel, dst-sharded, two SPMD launches, self-loop-as-slot):
  * Host prep (index ops only): CSR-sort edges by dst, shard nodes over the 8
    cores, bucket each core's nodes by (in-degree+1) -- the +1 is a self-loop
    slot prepended to every node's neighbor list -- and give every node a
    fixed number of edge slots (bucket stride).  Slot streams are
    host-gathered into the slot layout (bf16); padding slots carry zeros.
  * Launch A (per core): w = rsqrt(degS) per slot (degS = deg[src]+1, uint8);
    y = x[src]*w; dense fixed-stride reduce over slots -> agg (the self slot
    makes agg == sum_{j in N(i) u {i}} dinv_j x_j directly); per node:
    t_f = dinv^2 * agg_f;  g2 = relu([t0,t1,dinv] @ [W1;b1])  (== dinv *
    layer-1 output, the complete layer-2 message);  m = (g2 @ W2) * dinv
    written out per core in bf16 (3 features).
  * Host: concatenates the per-core m slices and gathers m[src] (incl. the
    self slot) into the slot layout for each core (index-space only).
  * Launch B (per core): dense fixed-stride reduce of the m slot stream ->
    agg2 (3 features, self included); h2 = sigmoid(dinv*agg2 + b2); then the
    MLP chain with weights baked as immediates: relu(.W3+b3) -> relu(.W4+b4)
    -> .W5+b5.
  * Host: unpermute per-core outputs back to original node order.

All floating-point arithmetic runs on device; the host only sorts, indexes,
pads, concatenates and casts dtypes.
"""
import numpy as np
import ml_dtypes

import concourse.bass as bass
from concourse.bacc import Bacc
import concourse.mybir as mybir
import concourse.tile as tile

NCORES = 8
N = 1_000_000
P = 128
F32 = mybir.dt.float32
BF16 = mybir.dt.bfloat16
U8 = mybir.dt.uint8
AF = mybir.ActivationFunctionType
OP = mybir.AluOpType
BF = ml_dtypes.bfloat16


# ----------------------------------------------------------------- host prep
def _choose_strides(max_need):
    ss = [s for s in (1, 2, 3, 4, 6, 8, 10, 12, 14, 16, 18, 20, 22, 24, 26,
                      28, 30, 32, 36, 40, 44, 48, 56, 64, 96, 128, 192, 256,
                      384, 512) if s < max_need]
    ss.append(int(max_need))
    return ss


def _prep(x, edge_index, ncores=NCORES, n=N):
    npc = n // ncores
    src = np.asarray(edge_index[0]).astype(np.int64)
    dst = np.asarray(edge_index[1]).astype(np.int64)
    deg_in = np.bincount(dst, minlength=n)
    need = deg_in + 1                              # self slot included
    strides = _choose_strides(max(int(need.max()), 2))
    strides_arr = np.asarray(strides)
    nb = len(strides)

    order = np.argsort(dst, kind="stable")
    src_s = src[order]
    rowptr = np.zeros(n + 1, dtype=np.int64)
    np.cumsum(deg_in, out=rowptr[1:])

    bucket_of = np.searchsorted(strides_arr, need)

    m_b = np.zeros((ncores, nb), dtype=np.int64)
    node_lists = [[None] * nb for _ in range(ncores)]
    for c in range(ncores):
        lo, hi = c * npc, (c + 1) * npc
        nodes_c = np.arange(lo, hi)
        bk = bucket_of[lo:hi]
        for b in range(nb):
            nl = nodes_c[bk == b]
            node_lists[c][b] = nl
            m_b[c, b] = -(-len(nl) // P)
    m_pad = m_b.max(axis=0)
    SUM_M_raw = int(m_pad.sum())
    SUM_M = -(-SUM_M_raw // 32) * 32
    # grow the biggest bucket to absorb the rounding (keeps layout math exact)
    m_pad[int(np.argmax(m_pad))] += SUM_M - SUM_M_raw
    NPCP = P * SUM_M
    boff = np.concatenate([[0], np.cumsum(m_pad)]).astype(np.int64)
    SLOTS = int((m_pad * P * strides_arr).sum())

    def make_plan(target):
        cp = []
        for b in range(nb):
            s = strides[b]
            if m_pad[b] == 0:
                continue
            mc = max(32, -(-max(1, target // s) // 32) * 32)
            i = 0
            while i < m_pad[b]:
                take = int(min(mc, m_pad[b] - i))
                cp.append((b, s, int(i), take))
                i += take
        return cp

    storage = np.empty(n, dtype=np.int64)
    origin = np.full((ncores, NPCP), -1, dtype=np.int64)
    for c in range(ncores):
        for b in range(nb):
            nl, mb, off = node_lists[c][b], int(m_pad[b]), int(boff[b])
            if len(nl) == 0 or mb == 0:
                continue
            j = np.arange(len(nl))
            p, i = j // mb, j % mb
            sid = p * SUM_M + off + i
            storage[nl] = c * NPCP + sid
            origin[c, sid] = nl

    per_core = []
    for c in range(ncores):
        xgb = np.zeros((SLOTS * 2,), dtype=BF)
        degS = np.ones((SLOTS,), dtype=np.uint8)
        idxs = np.full((SLOTS,), ncores * NPCP, dtype=np.int64)  # pad row
        sbase = 0
        for b in range(nb):
            s, mb = strides[b], int(m_pad[b])
            if mb == 0:
                continue
            nl = node_lists[c][b]
            if len(nl) > 0:
                j = np.arange(len(nl))
                p, i = j // mb, j % mb
                nd = need[nl]                      # 1 + deg slots per node
                node_rep = np.repeat(j, nd)
                k_in = np.arange(len(node_rep)) - np.repeat(
                    np.concatenate([[0], np.cumsum(nd)[:-1]]), nd)
                # slot value source: k_in==0 -> the node itself (self loop),
                # else CSR neighbor (k_in-1)
                sv = np.where(
                    k_in == 0, np.repeat(nl, nd),
                    src_s[np.minimum(np.repeat(rowptr[nl], nd)
                                     + np.maximum(k_in - 1, 0),
                                     len(src_s) - 1)])
                slot = sbase + p[node_rep] * (mb * s) + i[node_rep] * s + k_in
                slot_fm = sbase * 2 + (p[node_rep] * mb + i[node_rep]) * (2 * s) + k_in
                xgb[slot_fm] = x[sv, 0].astype(BF)
                xgb[slot_fm + s] = x[sv, 1].astype(BF)
                degS[slot] = np.minimum(need[sv], 255).astype(np.uint8)
                idxs[slot] = storage[sv]
            sbase += P * mb * s
        assert sbase == SLOTS

        deg_own = np.ones((NPCP,), dtype=np.float32)
        valid = origin[c] >= 0
        ov = origin[c][valid]
        deg_own[valid] = need[ov].astype(np.float32)
        per_core.append(dict(xgb=xgb, degS=degS, idxs=idxs, deg_own=deg_own))

    meta = dict(strides=strides, m_pad=m_pad, SUM_M=SUM_M, NPCP=NPCP,
                boff=boff, SLOTS=SLOTS,
                chunk_plan=make_plan(4096), chunk_plan_B=make_plan(4096),
                origin=origin, ncores=ncores, n=n)
    return per_core, meta


# ------------------------------------------------------- device build: utils
def _sbases(meta):
    sbases, sb = {}, 0
    for b, s in enumerate(meta["strides"]):
        sbases[b] = sb
        sb += P * int(meta["m_pad"][b]) * s
    return sbases


def _make_dinv(nc, dinv, deg_own):
    """dinv <- 1/sqrt(deg_own). deg_own f32 in DRAM, dinv [P, SUM_M] tile."""
    nc.sync.dma_start(out=dinv[:],
                      in_=deg_own[:].rearrange("(p j) -> p j", p=P))
    nc.vector.reciprocal_approx_fast(out=dinv[:], in_=dinv[:])
    nc.scalar.activation(out=dinv[:], in_=dinv[:], func=AF.Sqrt)


# --------------------------------------------------------- device build: A
def _build_A(meta, W1b, W2, reps=1):
    SUM_M, SLOTS, NPCP = meta["SUM_M"], meta["SLOTS"], meta["NPCP"]
    m_pad, boff = meta["m_pad"], meta["boff"]
    plan = meta["chunk_plan"]
    sbases = _sbases(meta)

    nc = Bacc(num_devices=meta["ncores"])
    xgb = nc.declare_dram_parameter("xgb", [SLOTS * 2], BF16, isOutput=False)
    degS = nc.declare_dram_parameter("degS", [SLOTS], U8, isOutput=False)
    deg_own = nc.declare_dram_parameter("deg_own", [NPCP], F32, isOutput=False)
    mout = nc.declare_dram_parameter("mout", [P, SUM_M, 3], BF16, isOutput=True)

    with tile.TileContext(nc) as tc:
        with tc.tile_pool(name="res", bufs=1) as res:
            dinv = res.tile([P, SUM_M], F32, tag="dinv")
            agg = res.tile([P, SUM_M, 2], F32, tag="agg")
            mst = res.tile([P, SUM_M, 3], BF16, tag="mst")
            for _ in range(reps):
                _make_dinv(nc, dinv, deg_own)

                with tc.tile_pool(name="l1", bufs=2) as st:
                    for (b, s, i0, mc) in plan:
                        mb = int(m_pad[b])
                        xv = xgb[2 * sbases[b]:2 * (sbases[b] + P * mb * s)] \
                            .rearrange("(p i fk) -> p i fk", p=P, i=mb)[:, i0:i0 + mc, :]
                        dv = degS[sbases[b]:sbases[b] + P * mb * s] \
                            .rearrange("(p i k) -> p i k", p=P, i=mb, k=s)[:, i0:i0 + mc, :]
                        xt = st.tile([P, mc, 2 * s], BF16, tag="xg")
                        wu = st.tile([P, mc, s], U8, tag="wu")
                        wf = st.tile([P, mc, s], F32, tag="wf")
                        wb = st.tile([P, mc, s], BF16, tag="wb")
                        nc.sync.dma_start(out=xt[:], in_=xv)
                        nc.sync.dma_start(out=wu[:], in_=dv)
                        nc.vector.tensor_scalar_mul(out=wf[:], in0=wu[:], scalar1=1.0)
                        nc.vector.reciprocal_approx_fast(out=wf[:], in_=wf[:])
                        nc.scalar.activation(out=wb[:], in_=wf[:], func=AF.Sqrt)

                        y = st.tile([P, mc, 2, s], BF16, tag="y")
                        nc.vector.tensor_tensor(out=y[:, :, 0, :],
                                                in0=xt[:, :, 0:s],
                                                in1=wb[:], op=OP.mult)
                        nc.gpsimd.tensor_tensor(out=y[:, :, 1, :],
                                                in0=xt[:, :, s:2 * s],
                                                in1=wb[:], op=OP.mult)
                        j0 = int(boff[b]) + i0
                        nc.vector.tensor_reduce(out=agg[:, j0:j0 + mc, :],
                                                in_=y[:],
                                                axis=mybir.AxisListType.X, op=OP.add)

                # per-node phase, full-width passes over [P, SUM_M]
                t0 = res.tile([P, SUM_M], F32, tag="t0")
                t1 = res.tile([P, SUM_M], F32, tag="t1")
                nc.vector.tensor_tensor(out=t0[:], in0=agg[:, :, 0], in1=dinv[:], op=OP.mult)
                nc.gpsimd.tensor_tensor(out=t1[:], in0=agg[:, :, 1], in1=dinv[:], op=OP.mult)
                nc.vector.tensor_tensor(out=t0[:], in0=t0[:], in1=dinv[:], op=OP.mult)
                nc.gpsimd.tensor_tensor(out=t1[:], in0=t1[:], in1=dinv[:], op=OP.mult)

                # g2_o = relu(t0 W1b[0,o] + t1 W1b[1,o] + dinv W1b[2,o])
                g2 = res.tile([P, SUM_M, 4], F32, tag="g2")
                for o in range(4):
                    eng = nc.vector
                    eng.tensor_scalar_mul(
                        out=g2[:, :, o], in0=t0[:], scalar1=float(W1b[0, o]))
                    eng.scalar_tensor_tensor(
                        out=g2[:, :, o], in0=t1[:], scalar=float(W1b[1, o]),
                        in1=g2[:, :, o], op0=OP.mult, op1=OP.add)
                    eng.scalar_tensor_tensor(
                        out=g2[:, :, o], in0=dinv[:], scalar=float(W1b[2, o]),
                        in1=g2[:, :, o], op0=OP.mult, op1=OP.add)
                nc.scalar.activation(out=g2[:], in_=g2[:], func=AF.Relu)

                # m_o = (g2 @ W2)_o  (dinv factor already inside g2)
                tmp = res.tile([P, SUM_M], F32, tag="tmp")
                for o in range(3):
                    eng = nc.vector
                    eng.tensor_scalar_mul(
                        out=tmp[:], in0=g2[:, :, 0], scalar1=float(W2[0, o]))
                    for f in range(1, 3):
                        eng.scalar_tensor_tensor(
                            out=tmp[:], in0=g2[:, :, f], scalar=float(W2[f, o]),
                            in1=tmp[:], op0=OP.mult, op1=OP.add)
                    eng.scalar_tensor_tensor(
                        out=mst[:, :, o], in0=g2[:, :, 3],
                        scalar=float(W2[3, o]), in1=tmp[:],
                        op0=OP.mult, op1=OP.add)
                nc.sync.dma_start(out=mout[:], in_=mst[:])
    return nc


# --------------------------------------------------------- device build: B
def _build_B(meta, weights, reps=1):
    SUM_M, SLOTS, NPCP = meta["SUM_M"], meta["SLOTS"], meta["NPCP"]
    m_pad, boff = meta["m_pad"], meta["boff"]
    W3, b3 = weights["W3"], weights["b3"]
    W4, b4 = weights["W4"], weights["b4"]
    W5, b5 = weights["W5"], weights["b5"]
    b2 = weights["b2"]
    plan = meta["chunk_plan_B"]
    sbases = _sbases(meta)

    nc = Bacc(num_devices=meta["ncores"])
    ms = nc.declare_dram_parameter("ms", [SLOTS * 3], BF16, isOutput=False)
    deg_own = nc.declare_dram_parameter("deg_own", [NPCP], F32, isOutput=False)
    out = nc.declare_dram_parameter("out", [P, SUM_M], F32, isOutput=True)

    with tile.TileContext(nc) as tc:
        with tc.tile_pool(name="res", bufs=1) as res:
            dinv = res.tile([P, SUM_M], F32, tag="dinv")
            h2 = [res.tile([P, SUM_M], F32, tag=f"h2_{o}", name=f"h2_{o}")
                  for o in range(3)]
            for _ in range(reps):
                _make_dinv(nc, dinv, deg_own)

                with tc.tile_pool(name="l2", bufs=2) as st:
                    for (b, s, i0, mc) in plan:
                        mb = int(m_pad[b])
                        gv = ms[3 * sbases[b]:3 * (sbases[b] + P * mb * s)] \
                            .rearrange("(p i fk) -> p i fk", p=P, i=mb)[:, i0:i0 + mc, :]
                        gt = st.tile([P, mc, 3 * s], BF16, tag="gath")
                        nc.sync.dma_start(out=gt[:], in_=gv)
                        a2 = st.tile([P, mc, 3], F32, tag="agg2")
                        nc.vector.tensor_reduce(
                            out=a2[:],
                            in_=gt[:].rearrange("p i (f k) -> p i f k", k=s),
                            axis=mybir.AxisListType.X, op=OP.add)
                        j0 = int(boff[b]) + i0
                        dsl = dinv[:, j0:j0 + mc]
                        for o in range(3):
                            eng = nc.vector if o % 2 == 0 else nc.gpsimd
                            eng.tensor_tensor(out=h2[o][:, j0:j0 + mc],
                                              in0=a2[:, :, o], in1=dsl, op=OP.mult)
                    for o in range(3):
                        bt = res.tile([P, 1], F32, tag=f"b2_{o}", name=f"b2_{o}")
                        nc.vector.memset(bt[:], float(b2[o]))
                        nc.scalar.activation(out=h2[o][:], in_=h2[o][:],
                                             func=AF.Sigmoid, bias=bt[:])

                # MLP with immediates
                def dense(ins_, Wm, bias, func, tagp, och):
                    outs_ = []
                    for o in range(och):
                        eng = nc.vector
                        acc = res.tile([P, SUM_M], F32, tag=f"{tagp}{o}",
                                       name=f"{tagp}{o}")
                        eng.tensor_scalar_mul(out=acc[:], in0=ins_[0][:],
                                              scalar1=float(Wm[0, o]))
                        for i in range(1, len(ins_)):
                            eng.scalar_tensor_tensor(
                                out=acc[:], in0=ins_[i][:], scalar=float(Wm[i, o]),
                                in1=acc[:], op0=OP.mult, op1=OP.add)
                        if func is not None:
                            bt = res.tile([P, 1], F32, tag=f"{tagp}b{o}",
                                          name=f"{tagp}b{o}")
                            nc.vector.memset(bt[:], float(bias[o]))
                            nc.scalar.activation(out=acc[:], in_=acc[:], func=func,
                                                 bias=bt[:])
                        else:
                            nc.vector.tensor_scalar_add(out=acc[:], in0=acc[:],
                                                        scalar1=float(bias[o]))
                        outs_.append(acc)
                    return outs_

                h = dense(h2, W3, b3, AF.Relu, "h3_", 4)
                h = dense(h, W4, b4, AF.Relu, "h4_", 3)
                h = dense(h, W5, b5, None, "h5_", 1)
                nc.sync.dma_start(out=out[:], in_=h[0][:])
    return nc


# ------------------------------------------------------------------ driver
def host_gather_m(meta, per_core, m_slices):
    """m_slices[c]: [P, SUM_M, 3] bf16 from launch A. Returns per-core slot
    streams [SLOTS*3] in f-major slot layout (index-space gather only)."""
    NC, NPCP = meta["ncores"], meta["NPCP"]
    strides, m_pad = meta["strides"], meta["m_pad"]
    m_full = np.concatenate(
        [np.asarray(m_slices[c]).reshape(NPCP, 3) for c in range(NC)] +
        [np.zeros((1, 3), np.asarray(m_slices[0]).dtype)], axis=0)
    out = []
    for c in range(NC):
        g = m_full[per_core[c]["idxs"]]          # [SLOTS, 3] slot-major
        fm = np.empty((meta["SLOTS"] * 3,), dtype=m_full.dtype)
        sbase = 0
        for b, s in enumerate(strides):
            mb = int(m_pad[b])
            if mb == 0:
                continue
            nseg = P * mb * s
            seg = g[sbase:sbase + nseg].reshape(P * mb, s, 3)
            fm[sbase * 3:(sbase + nseg) * 3] = \
                seg.transpose(0, 2, 1).reshape(-1)
            sbase += nseg
        out.append(fm)
    return out


def _run_spmd(nc, in_maps, ncores):
    from concourse.bass_utils import run_bass_kernel_spmd
    if not nc.is_finalized():
        nc.finalize()
    return run_bass_kernel_spmd(nc, in_maps, core_ids=list(range(ncores)))


def kernel(x, edge_index, W1, b1, W2, b2, W3, b3, W4, b4, W5, b5):
    x = np.asarray(x, dtype=np.float32)
    per_core, meta = _prep(x, edge_index)
    W1b = np.concatenate([np.asarray(W1), np.asarray(b1)[None, :]], axis=0)
    weights = dict(W2=np.asarray(W2), b2=np.asarray(b2),
                   W3=np.asarray(W3), b3=np.asarray(b3),
                   W4=np.asarray(W4), b4=np.asarray(b4),
                   W5=np.asarray(W5), b5=np.asarray(b5))
    NC = meta["ncores"]

    ncA = _build_A(meta, W1b, np.asarray(W2))
    resA = _run_spmd(ncA, [{k: d[k] for k in ("xgb", "degS", "deg_own")}
                           for d in per_core], NC)
    m_slices = [resA.results[c]["mout"] for c in range(NC)]

    ms = host_gather_m(meta, per_core, m_slices)
    ncB = _build_B(meta, weights)
    resB = _run_spmd(ncB, [dict(ms=ms[c], deg_own=per_core[c]["deg_own"])
                           for c in range(NC)], NC)

    full = np.zeros(meta["n"], dtype=np.float32)
    for c in range(NC):
        o = np.asarray(resB.results[c]["out"]).reshape(-1)
        org = meta["origin"][c]
        valid = org >= 0
        full[org[valid]] = o[valid]
    return full
